# revision 6
# baseline (speedup 1.0000x reference)
"""Contrastive projection head loss on 8 Trainium2 NeuronCores.

Reference computation (B=8192, E=1024, P=512):
    z_codon = relu(x[:, :E]) @ w + b          # [B, P]
    z_amino = relu(x[:, E:]) @ w + b          # [B, P]
    z  = concat([z_codon, z_amino], axis=1)   # [B, 2P]
    zn = z / max(||z||, 1e-8)
    s  = (zn @ zn.T);  s[i,i] = -9e15;  s /= 0.1
    nll_i = -s[i, (i - B/2) % B] + logsumexp(s[i, :])
    out = mean(nll)

Distribution: data-parallel over B (1024 rows/core). Each core projects and
normalizes its rows (z kept feature-major, i.e. z^T, so it is directly the
K-major operand of the similarity GEMM), all-gathers zn^T, then computes its
[1024 x 8192] block of the cosine-similarity matrix blockwise with a fused
exp+row-sum epilogue. The diagonal (self-similarity) term is removed by
subtracting its exp from the row sum, and the positive-pair logit is pulled
from the block diagonal of the (c+4) mod 8 column block; both are selected by
tiny per-core 0/1 input masks so the SPMD program is identical on all cores.
Matmuls run in float32r (full-rate fp32 on the PE array).

Returns per-core partial sums [1, 8]; host sums and divides by B.
"""
import numpy as np

from concourse import mybir, tile, bacc
from concourse.bass_utils import run_bass_kernel_spmd
from concourse.masks import make_identity

N_CORES = 8
B = 8192
E = 1024          # embedding size (per half)
P = 512           # projection size
D = 2 * P         # z feature dim = 1024
R = B // N_CORES  # rows per core = 1024
KT = D // 128     # feature sub-tiles = 8
MT = R // 128     # row sub-tiles per core = 8
CT = B // 512     # global column tiles of 512 = 16
INV_T = 10.0      # 1 / temperature
EPS = 1e-8

F32 = mybir.dt.float32
F32R = mybir.dt.float32r
AF = mybir.ActivationFunctionType
ALU = mybir.AluOpType

_cached = {}


def _build(no_collective=False, stop_after="full"):
    nc = bacc.Bacc("TRN2", target_bir_lowering=False, debug=False,
                   enable_asserts=False, num_devices=N_CORES)
    x_in = nc.dram_tensor("xs", [R, 2 * E], F32, kind="ExternalInput").ap()
    w_in = nc.dram_tensor("w", [E, P], F32, kind="ExternalInput").ap()
    b_in = nc.dram_tensor("b", [P], F32, kind="ExternalInput").ap()
    dsel_in = nc.dram_tensor("dsel", [128, MT, CT], F32, kind="ExternalInput").ap()
    psel_in = nc.dram_tensor("psel", [128, MT, CT], F32, kind="ExternalInput").ap()
    out = nc.dram_tensor("out", [1, MT], F32, kind="ExternalOutput").ap()

    with tile.TileContext(nc) as tc:
        with tc.tile_pool(name="const", bufs=1) as const, \
             tc.tile_pool(name="big", bufs=2) as big, \
             tc.tile_pool(name="stage", bufs=1) as stage, \
             tc.tile_pool(name="small", bufs=1) as small, \
             tc.tile_pool(name="dram", bufs=1, space="DRAM") as dram:

            ident = const.tile([128, 128], F32)
            make_identity(nc, ident[:])
            ones_f = const.tile([128, 1], F32)
            nc.vector.memset(ones_f[:], 1.0)
            ones_r = const.tile([128, 1], F32R)
            nc.scalar.copy(ones_r[:], ones_f[:])
            b2 = const.tile([128, P // 128], F32)
            nc.sync.dma_start(b2[:], b_in.rearrange("(mt p) -> p mt", p=128))
            dsel = const.tile([128, MT, CT], F32)
            nc.sync.dma_start(dsel[:], dsel_in[:])
            psel = const.tile([128, MT, CT], F32)
            nc.sync.dma_start(psel[:], psel_in[:])

            # w as [128, KT(=E/128), P] float32r
            wstage = stage.tile([128, E // 128, P], F32, tag="wstage")
            nc.sync.dma_start(wstage[:], w_in.rearrange("(kt p) q -> p kt q", p=128))
            w_r = const.tile([128, E // 128, P], F32R)
            nc.vector.tensor_copy(w_r[:], wstage[:])

            # ---- phase 1: transpose x halves (+relu), project to z^T ----
            xT = big.tile([128, KT, R], F32R, tag="big")   # relu(x)^T per half
            zT = big.tile([128, KT, R], F32, tag="big")    # z^T (feature-major)

            with tc.tile_pool(name="xrow", bufs=3) as xrowp, \
                 tc.tile_pool(name="ps1", bufs=2, space="PSUM") as ps1:
                for h in range(2):
                    # transpose half h: xT[:, h*4..] holds relu(x_h)^T [E, R]
                    for r in range(MT):
                        xrow = xrowp.tile([128, E], F32, tag="xrow")
                        nc.sync.dma_start(
                            xrow[:], x_in[r * 128:(r + 1) * 128,
                                          h * E:(h + 1) * E])
                        for ct in range(E // 128):
                            pt = ps1.tile([128, 128], F32, tag="tp", bufs=3)
                            nc.tensor.transpose(
                                pt[:], xrow[:, ct * 128:(ct + 1) * 128], ident[:])
                            nc.scalar.activation(
                                xT[:, ct, r * 128:(r + 1) * 128], pt[:], AF.Relu)
                    # project: zT[., h*4+m4, .] = w[:,m4]^T @ relu(x_h)^T + b
                    for m4 in range(P // 128):
                        for n2 in range(R // 512):
                            pz = ps1.tile([128, 512], F32, tag="pz", bufs=2)
                            for kt in range(E // 128):
                                nc.tensor.matmul(
                                    pz[:],
                                    w_r[:, kt, m4 * 128:(m4 + 1) * 128],
                                    xT[:, kt, n2 * 512:(n2 + 1) * 512],
                                    start=(kt == 0), stop=(kt == E // 128 - 1))
                            nc.scalar.activation(
                                zT[:, h * 4 + m4, n2 * 512:(n2 + 1) * 512],
                                pz[:], AF.Identity,
                                bias=b2[:, m4:m4 + 1], scale=1.0)

                # ---- phase 1b: row norms and normalization ----
                pn = [ps1.tile([1, 512], F32, tag=f"pn{i}", bufs=1,
                               name=f"pn{i}")
                      for i in range(2)]
                with tc.tile_pool(name="sqp", bufs=2) as sqp:
                    for kt in range(KT):
                        sq = sqp.tile([128, R], F32R, tag="sq")
                        nc.scalar.activation(sq[:], zT[:, kt, :], AF.Square)
                        for i in range(2):
                            nc.tensor.matmul(
                                pn[i][:], ones_r[:], sq[:, i * 512:(i + 1) * 512],
                                start=(kt == 0), stop=(kt == KT - 1))
                nrm = small.tile([1, R], F32)
                for i in range(2):
                    nc.scalar.activation(nrm[:, i * 512:(i + 1) * 512],
                                         pn[i][:], AF.Sqrt)
                nc.vector.tensor_scalar_max(nrm[:], nrm[:], EPS)
                rn_strip = small.tile([1, R], F32)
                nc.vector.reciprocal(rn_strip[:], nrm[:])
                rn_dram = dram.tile([R], F32)
                nc.sync.dma_start(rn_dram[None, :], rn_strip[:])
                rn_bc = const.tile([128, R], F32)
                nc.sync.dma_start(rn_bc[:], rn_dram[None, :].to_broadcast([128, R]))

            znT = big.tile([128, KT, R], F32R, tag="big")  # reuses xT's slot
            for kt in range(KT):
                nc.vector.tensor_tensor(znT[:, kt, :], zT[:, kt, :], rn_bc[:],
                                        ALU.mult)

            # ---- all-gather zn^T across the 8 cores ----
            ag_in = dram.tile([D, R], F32R)
            ag_out = dram.tile([N_CORES * D, R], F32R,
                               addr_space="Local" if no_collective else "Shared")
            nc.sync.dma_start(
                ag_in.rearrange("(kt p) j -> p kt j", p=128), znT[:])
            if no_collective:
                for c in range(N_CORES):
                    nc.sync.dma_start(ag_out[c * D:(c + 1) * D, :], ag_in[:])
            else:
                nc.gpsimd.collective_compute(
                    "AllGather", ALU.bypass,
                    replica_groups=[list(range(N_CORES))],
                    ins=[ag_in[:]], outs=[ag_out[:]])

            # ---- phase 2: blockwise cos-sim + fused exp/rowsum ----
            rowsum = const.tile([128, MT, CT], F32)
            pos_acc = const.tile([128, MT], F32)
            corr_acc = const.tile([128, MT], F32)
            nc.vector.memset(pos_acc[:], 0.0)
            nc.vector.memset(corr_acc[:], 0.0)

            with tc.tile_pool(name="rhs", bufs=3) as rhsp, \
                 tc.tile_pool(name="junk", bufs=2) as junkp, \
                 tc.tile_pool(name="dtmp", bufs=4) as dtmpp, \
                 tc.tile_pool(name="ps2", bufs=1, space="PSUM") as ps2:
                for ci in range(CT):
                    c2, nb = ci // 2, ci % 2
                    rhs = rhsp.tile([128, KT, 512], F32R, tag="rhs")
                    src = ag_out[c2 * D:(c2 + 1) * D, :].rearrange(
                        "(kt p) j -> p kt j", p=128)[:, :, nb * 512:(nb + 1) * 512]
                    nc.sync.dma_start(rhs[:], src)
                    for m in range(MT):
                        pg = ps2.tile([128, 512], F32, tag="pg", bufs=6)
                        for kt in range(KT):
                            nc.tensor.matmul(
                                pg[:], znT[:, kt, m * 128:(m + 1) * 128],
                                rhs[:, kt, :],
                                start=(kt == 0), stop=(kt == KT - 1))
                        junk = junkp.tile([128, 512], F32, tag="junk")
                        nc.scalar.activation(
                            junk[:], pg[:], AF.Exp, scale=INV_T,
                            accum_out=rowsum[:, m, ci:ci + 1])
                        # diagonal / positive-pair handling
                        if ci >= m // 4 and (ci - m // 4) % 2 == 0:
                            off = (m % 4) * 128
                            jd = junkp.tile([128, 128], F32, tag="jd")
                            nc.vector.tensor_tensor(
                                jd[:], pg[:, off:off + 128], ident[:],
                                ALU.mult)
                            d = dtmpp.tile([128, 1], F32, tag="d")
                            nc.vector.reduce_sum(d[:], jd[:],
                                                 axis=mybir.AxisListType.X)
                            tp = dtmpp.tile([128, 1], F32, tag="tp2")
                            nc.vector.tensor_scalar(
                                tp[:], d[:], psel[:, m, ci:ci + 1], INV_T,
                                ALU.mult, ALU.mult)
                            nc.vector.tensor_tensor(
                                pos_acc[:, m:m + 1], pos_acc[:, m:m + 1],
                                tp[:], ALU.add)
                            ed = dtmpp.tile([128, 1], F32, tag="ed")
                            nc.scalar.activation(ed[:], d[:], AF.Exp,
                                                 scale=INV_T)
                            tc2 = dtmpp.tile([128, 1], F32, tag="tc2")
                            nc.vector.tensor_scalar(
                                tc2[:], ed[:], dsel[:, m, ci:ci + 1], None,
                                ALU.mult)
                            nc.vector.tensor_tensor(
                                corr_acc[:, m:m + 1], corr_acc[:, m:m + 1],
                                tc2[:], ALU.add)

                # ---- finale: lse, nll, partial sum ----
                nll = small.tile([128, MT], F32)
                for m in range(MT):
                    rs = dtmpp.tile([128, 1], F32, tag="rs")
                    nc.vector.reduce_sum(rs[:], rowsum[:, m, :],
                                         axis=mybir.AxisListType.X)
                    nc.vector.tensor_tensor(rs[:], rs[:],
                                            corr_acc[:, m:m + 1], ALU.subtract)
                    lse = dtmpp.tile([128, 1], F32, tag="lse")
                    nc.scalar.activation(lse[:], rs[:], AF.Ln)
                    nc.vector.tensor_tensor(nll[:, m:m + 1], lse[:],
                                            pos_acc[:, m:m + 1], ALU.subtract)
                pf = ps2.tile([1, MT], F32, tag="pf", bufs=1)
                nc.tensor.matmul(pf[:], ones_f[:], nll[:], start=True, stop=True)
                fs = small.tile([1, MT], F32)
                nc.scalar.copy(fs[:], pf[:])
                nc.sync.dma_start(out[:], fs[:])

    nc.compile()
    return nc


def _sel_masks(c):
    dsel = np.zeros((128, MT, CT), dtype=np.float32)
    psel = np.zeros((128, MT, CT), dtype=np.float32)
    for m in range(MT):
        dsel[:, m, 2 * c + m // 4] = 1.0
        psel[:, m, 2 * ((c + 4) % N_CORES) + m // 4] = 1.0
    return dsel, psel


def kernel(x, w, b):
    if "nc" not in _cached:
        _cached["nc"] = _build()
    nc = _cached["nc"]
    x = np.ascontiguousarray(np.asarray(x, dtype=np.float32))
    w = np.ascontiguousarray(np.asarray(w, dtype=np.float32))
    b = np.ascontiguousarray(np.asarray(b, dtype=np.float32))
    in_maps = []
    for c in range(N_CORES):
        dsel, psel = _sel_masks(c)
        in_maps.append({
            "xs": np.ascontiguousarray(x[c * R:(c + 1) * R]),
            "w": w, "b": b, "dsel": dsel, "psel": psel,
        })
    res = run_bass_kernel_spmd(nc, in_maps, list(range(N_CORES)))
    total = 0.0
    for c in range(N_CORES):
        total += float(res.results[c]["out"].astype(np.float64).sum())
    return np.float32(total / B)


# revision 7
# speedup vs baseline: 1.1016x; 1.1016x over previous
"""Contrastive projection head loss on 8 Trainium2 NeuronCores.

Reference computation (B=8192, E=1024, P=512):
    z_codon = relu(x[:, :E]) @ w + b          # [B, P]
    z_amino = relu(x[:, E:]) @ w + b          # [B, P]
    z  = concat([z_codon, z_amino], axis=1)   # [B, 2P]
    zn = z / max(||z||, 1e-8)
    s  = (zn @ zn.T);  s[i,i] = -9e15;  s /= 0.1
    nll_i = -s[i, (i - B/2) % B] + logsumexp(s[i, :])
    out = mean(nll)

Distribution: data-parallel over B (1024 rows/core). Each core projects and
normalizes its rows (kept feature-major as zn^T — directly the K-major
operand of the similarity GEMM), all-gathers zn^T in two column chunks, and
computes its [1024 x 8192] block of the cosine-similarity matrix blockwise
with a fused exp+row-sum epilogue (ACT accum_out).

Latency hiding: the local diagonal block is computed straight from SBUF
while the collectives fly, and its duplicate contribution from the gathered
pass is subtracted via a per-core 0/1 mask ("osel") so the SPMD program is
identical on all cores. The self-similarity term is removed by subtracting
its exp (extracted from the local block at compile-time-known positions);
the positive-pair logit is pulled from the block diagonal of the
(c+4) mod 8 column block, selected by the per-core "psel" mask.
Matmuls run in float32r (full-rate fp32 on the PE array).

Returns per-core partial sums [1, 8]; host sums and divides by B.
"""
import numpy as np

from concourse import mybir, tile, bacc
from concourse.bass_utils import run_bass_kernel_spmd
from concourse.masks import make_identity

N_CORES = 8
B = 8192
E = 1024          # embedding size (per half)
P = 512           # projection size
D = 2 * P         # z feature dim = 1024
R = B // N_CORES  # rows per core = 1024
KT = D // 128     # feature sub-tiles = 8
MT = R // 128     # row sub-tiles per core = 8
CT = B // 512     # global column tiles of 512 = 16
INV_T = 10.0      # 1 / temperature
EPS = 1e-8

F32 = mybir.dt.float32
F32R = mybir.dt.float32r
BF16 = mybir.dt.bfloat16
AF = mybir.ActivationFunctionType
ALU = mybir.AluOpType

GATHER_DT = F32R  # dtype of the gathered zn^T (F32R or BF16)

_cached = {}


def _gemm_tile(nc, pg, lhsT_src, rhs_src, m):
    """8 accumulated matmuls: pg[128,512] += znT[:,kt,m-block].T @ rhs[:,kt,:]"""
    for kt in range(KT):
        nc.tensor.matmul(pg[:], lhsT_src[:, kt, m * 128:(m + 1) * 128],
                         rhs_src[:, kt, :],
                         start=(kt == 0), stop=(kt == KT - 1))


def _build(no_collective=False):
    gdt = GATHER_DT
    nc = bacc.Bacc("TRN2", target_bir_lowering=False, debug=False,
                   enable_asserts=False, num_devices=N_CORES)
    x_in = nc.dram_tensor("xs", [R, 2 * E], F32, kind="ExternalInput").ap()
    w_in = nc.dram_tensor("w", [E, P], F32, kind="ExternalInput").ap()
    b_in = nc.dram_tensor("b", [P], F32, kind="ExternalInput").ap()
    osel_in = nc.dram_tensor("osel", [128, CT], F32, kind="ExternalInput").ap()
    psel_in = nc.dram_tensor("psel", [128, MT, CT], F32, kind="ExternalInput").ap()
    out = nc.dram_tensor("out", [1, MT], F32, kind="ExternalOutput").ap()

    with tile.TileContext(nc) as tc:
        with tc.tile_pool(name="const", bufs=1) as const, \
             tc.tile_pool(name="big", bufs=2) as big, \
             tc.tile_pool(name="stage", bufs=1) as stage, \
             tc.tile_pool(name="small", bufs=1) as small, \
             tc.tile_pool(name="dram", bufs=1, space="DRAM") as dram:

            ident = const.tile([128, 128], F32)
            make_identity(nc, ident[:])
            ones_f = const.tile([128, 1], F32)
            nc.vector.memset(ones_f[:], 1.0)
            ones_r = const.tile([128, 1], F32R)
            nc.vector.tensor_copy(ones_r[:], ones_f[:])
            b2 = const.tile([128, P // 128], F32)
            nc.sync.dma_start(b2[:], b_in.rearrange("(mt p) -> p mt", p=128))
            osel = const.tile([128, CT], F32)
            nc.sync.dma_start(osel[:], osel_in[:])
            psel = const.tile([128, MT, CT], F32)
            nc.sync.dma_start(psel[:], psel_in[:])

            # w as [128, KT(=E/128), P] float32r
            wstage = stage.tile([128, E // 128, P], F32, tag="wstage")
            nc.sync.dma_start(wstage[:], w_in.rearrange("(kt p) q -> p kt q", p=128))
            w_r = const.tile([128, E // 128, P], F32R)
            nc.vector.tensor_copy(w_r[:], wstage[:])

            # ---- phase 1: transpose x halves (+relu), project to z^T ----
            xT = big.tile([128, KT, R], F32R, tag="big")   # relu(x)^T per half
            zT = big.tile([128, KT, R], F32, tag="big")    # z^T (feature-major)

            with tc.tile_pool(name="xrow", bufs=3) as xrowp, \
                 tc.tile_pool(name="ps1", bufs=2, space="PSUM") as ps1:
                for h in range(2):
                    # transpose half h: xT[:, h*4..] holds relu(x_h)^T [E, R]
                    for r in range(MT):
                        xrow = xrowp.tile([128, E], F32, tag="xrow")
                        nc.sync.dma_start(
                            xrow[:], x_in[r * 128:(r + 1) * 128,
                                          h * E:(h + 1) * E])
                        for ct in range(E // 128):
                            pt = ps1.tile([128, 128], F32, tag="tp", bufs=3)
                            nc.tensor.transpose(
                                pt[:], xrow[:, ct * 128:(ct + 1) * 128], ident[:])
                            # relu on DVE (keeps ACT tables quiet)
                            nc.vector.tensor_scalar_max(
                                xT[:, ct, r * 128:(r + 1) * 128], pt[:], 0.0)
                    # project: zT[., h*4+m4, .] = w[:,m4]^T @ relu(x_h)^T + b
                    for m4 in range(P // 128):
                        for n2 in range(R // 512):
                            pz = ps1.tile([128, 512], F32, tag="pz", bufs=2)
                            for kt in range(E // 128):
                                nc.tensor.matmul(
                                    pz[:],
                                    w_r[:, kt, m4 * 128:(m4 + 1) * 128],
                                    xT[:, kt, n2 * 512:(n2 + 1) * 512],
                                    start=(kt == 0), stop=(kt == E // 128 - 1))
                            # bias add on DVE, psum -> sbuf
                            nc.vector.tensor_scalar(
                                zT[:, h * 4 + m4, n2 * 512:(n2 + 1) * 512],
                                pz[:], b2[:, m4:m4 + 1], None, ALU.add)

                # ---- phase 1b: row norms and normalization ----
                pn = [ps1.tile([1, 512], F32, tag=f"pn{i}", bufs=1,
                               name=f"pn{i}")
                      for i in range(2)]
                with tc.tile_pool(name="sqp", bufs=2) as sqp:
                    for kt in range(KT):
                        sq = sqp.tile([128, R], F32R, tag="sq")
                        nc.vector.tensor_tensor(sq[:], zT[:, kt, :],
                                                zT[:, kt, :], ALU.mult)
                        for i in range(2):
                            nc.tensor.matmul(
                                pn[i][:], ones_r[:], sq[:, i * 512:(i + 1) * 512],
                                start=(kt == 0), stop=(kt == KT - 1))
                nrm = small.tile([1, R], F32)
                for i in range(2):
                    nc.scalar.activation(nrm[:, i * 512:(i + 1) * 512],
                                         pn[i][:], AF.Sqrt)
                nc.vector.tensor_scalar_max(nrm[:], nrm[:], EPS)
                rn_strip = small.tile([1, R], F32)
                nc.vector.reciprocal(rn_strip[:], nrm[:])
                rn_dram = dram.tile([R], F32)
                nc.sync.dma_start(rn_dram[None, :], rn_strip[:])
                rn_bc = const.tile([128, R], F32)
                nc.sync.dma_start(rn_bc[:], rn_dram[None, :].to_broadcast([128, R]))

            znT = big.tile([128, KT, R], gdt, tag="big")  # reuses xT's slot
            for kt in range(KT):
                nc.vector.tensor_tensor(znT[:, kt, :], zT[:, kt, :], rn_bc[:],
                                        ALU.mult)

            # ---- all-gather zn^T in two j-chunks of 512 rows each ----
            ag_in = [dram.tile([D, 512], gdt, name=f"ag_in{k}")
                     for k in range(2)]
            ag_out = [dram.tile([N_CORES * D, 512], gdt, name=f"ag_out{k}",
                                addr_space="Local" if no_collective else "Shared")
                      for k in range(2)]
            for k in range(2):
                nc.sync.dma_start(
                    ag_in[k].rearrange("(kt p) j -> p kt j", p=128),
                    znT[:, :, k * 512:(k + 1) * 512])
                if no_collective:
                    for c in range(N_CORES):
                        nc.sync.dma_start(ag_out[k][c * D:(c + 1) * D, :],
                                          ag_in[k][:])
                else:
                    nc.gpsimd.collective_compute(
                        "AllGather", ALU.bypass,
                        replica_groups=[list(range(N_CORES))],
                        ins=[ag_in[k][:]], outs=[ag_out[k][:]])

            # ---- phase 2: blockwise cos-sim + fused exp/rowsum ----
            # slots 0..15: gathered col-tiles; 16..17: local block (dup of
            # own two slots, subtracted via osel at the end)
            rowsum = const.tile([128, MT, CT + 2], F32)
            pos_acc = const.tile([128, MT], F32)
            corr_acc = const.tile([128, MT], F32)
            nc.vector.memset(pos_acc[:], 0.0)
            nc.vector.memset(corr_acc[:], 0.0)

            with tc.tile_pool(name="rhs", bufs=3) as rhsp, \
                 tc.tile_pool(name="junk", bufs=2) as junkp, \
                 tc.tile_pool(name="dtmp", bufs=4) as dtmpp, \
                 tc.tile_pool(name="ps2", bufs=1, space="PSUM") as ps2:

                # local block first — overlaps the collectives
                for nb in range(2):
                    for m in range(MT):
                        pg = ps2.tile([128, 512], F32, tag="pg", bufs=6,
                                      name=f"pgl{nb}_{m}")
                        _gemm_tile(nc, pg, znT,
                                   znT[:, :, nb * 512:(nb + 1) * 512], m)
                        junk = junkp.tile([128, 512], F32, tag="junk")
                        nc.scalar.activation(
                            junk[:], pg[:], AF.Exp, scale=INV_T,
                            accum_out=rowsum[:, m, CT + nb:CT + nb + 1])
                        if nb == m // 4:
                            # self-similarity at compile-time position
                            off = (m % 4) * 128
                            jd = junkp.tile([128, 128], F32, tag="jd")
                            nc.vector.tensor_tensor(
                                jd[:], pg[:, off:off + 128], ident[:],
                                ALU.mult)
                            d = dtmpp.tile([128, 1], F32, tag="d")
                            nc.vector.reduce_sum(d[:], jd[:],
                                                 axis=mybir.AxisListType.X)
                            nc.scalar.activation(
                                corr_acc[:, m:m + 1], d[:], AF.Exp,
                                scale=INV_T)

                # gathered blocks, chunk by chunk
                for k in range(2):
                    for c2 in range(N_CORES):
                        ci = 2 * c2 + k
                        rhs = rhsp.tile([128, KT, 512], gdt, tag="rhs")
                        src = ag_out[k][c2 * D:(c2 + 1) * D, :].rearrange(
                            "(kt p) j -> p kt j", p=128)
                        nc.sync.dma_start(rhs[:], src)
                        for m in range(MT):
                            pg = ps2.tile([128, 512], F32, tag="pg", bufs=6,
                                          name=f"pg{ci}_{m}")
                            _gemm_tile(nc, pg, znT, rhs, m)
                            junk = junkp.tile([128, 512], F32, tag="junk")
                            nc.scalar.activation(
                                junk[:], pg[:], AF.Exp, scale=INV_T,
                                accum_out=rowsum[:, m, ci:ci + 1])
                            # positive-pair logit (remote block diagonal)
                            if ci >= m // 4 and (ci - m // 4) % 2 == 0:
                                off = (m % 4) * 128
                                jd = junkp.tile([128, 128], F32, tag="jd")
                                nc.vector.tensor_tensor(
                                    jd[:], pg[:, off:off + 128], ident[:],
                                    ALU.mult)
                                d = dtmpp.tile([128, 1], F32, tag="d")
                                nc.vector.reduce_sum(d[:], jd[:],
                                                     axis=mybir.AxisListType.X)
                                tp = dtmpp.tile([128, 1], F32, tag="tp2")
                                nc.vector.tensor_scalar(
                                    tp[:], d[:], psel[:, m, ci:ci + 1], INV_T,
                                    ALU.mult, ALU.mult)
                                nc.vector.tensor_tensor(
                                    pos_acc[:, m:m + 1], pos_acc[:, m:m + 1],
                                    tp[:], ALU.add)

                # ---- finale: lse, nll, partial sum ----
                nll = small.tile([128, MT], F32)
                for m in range(MT):
                    rs = dtmpp.tile([128, 1], F32, tag="rs")
                    nc.vector.reduce_sum(rs[:], rowsum[:, m, :],
                                         axis=mybir.AxisListType.X)
                    own = dtmpp.tile([128, CT], F32, tag="own")
                    nc.vector.tensor_tensor(own[:], rowsum[:, m, :CT],
                                            osel[:], ALU.mult)
                    ro = dtmpp.tile([128, 1], F32, tag="ro")
                    nc.vector.reduce_sum(ro[:], own[:],
                                         axis=mybir.AxisListType.X)
                    nc.vector.tensor_tensor(rs[:], rs[:], ro[:], ALU.subtract)
                    nc.vector.tensor_tensor(rs[:], rs[:],
                                            corr_acc[:, m:m + 1], ALU.subtract)
                    lse = dtmpp.tile([128, 1], F32, tag="lse")
                    nc.scalar.activation(lse[:], rs[:], AF.Ln)
                    nc.vector.tensor_tensor(nll[:, m:m + 1], lse[:],
                                            pos_acc[:, m:m + 1], ALU.subtract)
                pf = ps2.tile([1, MT], F32, tag="pf", bufs=1)
                nc.tensor.matmul(pf[:], ones_f[:], nll[:], start=True, stop=True)
                fs = small.tile([1, MT], F32)
                nc.vector.tensor_copy(fs[:], pf[:])
                nc.sync.dma_start(out[:], fs[:])

    nc.compile()
    return nc


def _sel_masks(c):
    osel = np.zeros((128, CT), dtype=np.float32)
    osel[:, 2 * c] = 1.0
    osel[:, 2 * c + 1] = 1.0
    psel = np.zeros((128, MT, CT), dtype=np.float32)
    for m in range(MT):
        psel[:, m, 2 * ((c + 4) % N_CORES) + m // 4] = 1.0
    return osel, psel


def kernel(x, w, b):
    if "nc" not in _cached:
        _cached["nc"] = _build()
    nc = _cached["nc"]
    x = np.ascontiguousarray(np.asarray(x, dtype=np.float32))
    w = np.ascontiguousarray(np.asarray(w, dtype=np.float32))
    b = np.ascontiguousarray(np.asarray(b, dtype=np.float32))
    in_maps = []
    for c in range(N_CORES):
        osel, psel = _sel_masks(c)
        in_maps.append({
            "xs": np.ascontiguousarray(x[c * R:(c + 1) * R]),
            "w": w, "b": b, "osel": osel, "psel": psel,
        })
    res = run_bass_kernel_spmd(nc, in_maps, list(range(N_CORES)))
    total = 0.0
    for c in range(N_CORES):
        total += float(res.results[c]["out"].astype(np.float64).sum())
    return np.float32(total / B)


# revision 8
# speedup vs baseline: 1.1103x; 1.0079x over previous
"""Contrastive projection head loss on 8 Trainium2 NeuronCores.

Reference computation (B=8192, E=1024, P=512):
    z_codon = relu(x[:, :E]) @ w + b          # [B, P]
    z_amino = relu(x[:, E:]) @ w + b          # [B, P]
    z  = concat([z_codon, z_amino], axis=1)   # [B, 2P]
    zn = z / max(||z||, 1e-8)
    s  = (zn @ zn.T);  s[i,i] = -9e15;  s /= 0.1
    nll_i = -s[i, (i - B/2) % B] + logsumexp(s[i, :])
    out = mean(nll)

Distribution: data-parallel over B (1024 rows/core). Each core projects and
normalizes its rows (kept feature-major as zn^T — directly the K-major
operand of the similarity GEMM), all-gathers zn^T, then computes its
[1024 x 8192] block of the cosine-similarity matrix blockwise with a fused
exp+row-sum epilogue (ACT accum_out).

Latency hiding: phase 1 is pipelined by row-halves so the first of two
chunked AllGathers launches as soon as half the rows are projected and
normalized; the local diagonal block is computed straight from SBUF while
the collectives fly, and its duplicate contribution from the gathered pass
is subtracted via a per-core 0/1 mask ("osel") so the SPMD program stays
identical on all cores. The self-similarity term is removed by subtracting
its exp (extracted from the local block at compile-time-known positions);
the positive-pair logit is pulled from the block diagonal of the
(c+4) mod 8 column block, selected by the per-core "psel" mask.
Matmuls run in float32r (full-rate fp32 on the PE array).

Returns per-core partial sums [1, 8]; host sums and divides by B.
"""
import numpy as np

from concourse import mybir, tile, bacc
from concourse.bass_utils import run_bass_kernel_spmd
from concourse.masks import make_identity

N_CORES = 8
B = 8192
E = 1024          # embedding size (per half)
P = 512           # projection size
D = 2 * P         # z feature dim = 1024
R = B // N_CORES  # rows per core = 1024
KT = D // 128     # feature sub-tiles = 8
MT = R // 128     # row sub-tiles per core = 8
CT = B // 512     # global column tiles of 512 = 16
INV_T = 10.0      # 1 / temperature
EPS = 1e-8

F32 = mybir.dt.float32
F32R = mybir.dt.float32r
AF = mybir.ActivationFunctionType
ALU = mybir.AluOpType

_cached = {}


def _build(no_collective=False):
    nc = bacc.Bacc("TRN2", target_bir_lowering=False, debug=False,
                   enable_asserts=False, num_devices=N_CORES)
    x_in = nc.dram_tensor("xs", [R, 2 * E], F32, kind="ExternalInput").ap()
    w_in = nc.dram_tensor("w", [E, P], F32, kind="ExternalInput").ap()
    b_in = nc.dram_tensor("b", [P], F32, kind="ExternalInput").ap()
    osel_in = nc.dram_tensor("osel", [128, CT], F32, kind="ExternalInput").ap()
    psel_in = nc.dram_tensor("psel", [128, MT, CT], F32, kind="ExternalInput").ap()
    out = nc.dram_tensor("out", [1, MT], F32, kind="ExternalOutput").ap()

    with tile.TileContext(nc) as tc:
        with tc.tile_pool(name="const", bufs=1) as const, \
             tc.tile_pool(name="big", bufs=2) as big, \
             tc.tile_pool(name="small", bufs=1) as small, \
             tc.tile_pool(name="dram", bufs=1, space="DRAM") as dram:

            ident = const.tile([128, 128], F32)
            make_identity(nc, ident[:])
            ones_f = const.tile([128, 1], F32)
            nc.vector.memset(ones_f[:], 1.0)
            ones_r = const.tile([128, 1], F32R)
            nc.vector.tensor_copy(ones_r[:], ones_f[:])
            b2 = const.tile([128, P // 128], F32)
            nc.sync.dma_start(b2[:], b_in.rearrange("(mt p) -> p mt", p=128))
            osel = const.tile([128, CT], F32)
            nc.sync.dma_start(osel[:], osel_in[:])
            psel = const.tile([128, MT, CT], F32)
            nc.sync.dma_start(psel[:], psel_in[:])
            rn_bc = const.tile([128, R], F32)

            # w as [128, KT(=E/128), P] float32r — staged in a scoped pool
            w_r = const.tile([128, E // 128, P], F32R)
            with tc.tile_pool(name="wst", bufs=1) as wst:
                wstage = wst.tile([128, E // 128, P], F32, tag="wstage")
                nc.sync.dma_start(wstage[:],
                                  w_in.rearrange("(kt p) q -> p kt q", p=128))
                nc.vector.tensor_copy(w_r[:], wstage[:])

            # z^T feature-major, f32r; znT is the normalized copy
            zT = big.tile([128, KT, R], F32R, tag="z")
            znT = big.tile([128, KT, R], F32R, tag="z")
            ag_in = [dram.tile([D, 512], F32R, name=f"ag_in{k}")
                     for k in range(2)]
            ag_out = [dram.tile([N_CORES * D, 512], F32R, name=f"ag_out{k}",
                                addr_space="Local" if no_collective else "Shared")
                      for k in range(2)]
            rn_dram = dram.tile([R], F32)

            # ---- phase 1, pipelined over row-halves jh ----
            with tc.tile_pool(name="xrow", bufs=2) as xrowp, \
                 tc.tile_pool(name="xTp", bufs=2) as xTp, \
                 tc.tile_pool(name="sqp", bufs=2) as sqp, \
                 tc.tile_pool(name="ps1", bufs=2, space="PSUM") as ps1:
                for jh in range(2):
                    # transpose rows of this half (both x halves), with relu
                    xT = xTp.tile([128, 2 * KT, 512], F32R, tag="xT",
                                  name=f"xT{jh}")
                    for r in range(4):
                        rg = jh * 4 + r
                        xrow = xrowp.tile([128, 2 * E], F32, tag="xrow")
                        nc.sync.dma_start(xrow[:],
                                          x_in[rg * 128:(rg + 1) * 128, :])
                        for ct in range(2 * E // 128):
                            pt = ps1.tile([128, 128], F32, tag="tp", bufs=3)
                            nc.tensor.transpose(
                                pt[:], xrow[:, ct * 128:(ct + 1) * 128],
                                ident[:])
                            nc.vector.tensor_scalar_max(
                                xT[:, ct, r * 128:(r + 1) * 128], pt[:], 0.0)
                    # project this half: zT[:, h*4+m4, jh*512:...]
                    for h in range(2):
                        for m4 in range(P // 128):
                            pz = ps1.tile([128, 512], F32, tag="pz", bufs=2)
                            for kt in range(E // 128):
                                nc.tensor.matmul(
                                    pz[:],
                                    w_r[:, kt, m4 * 128:(m4 + 1) * 128],
                                    xT[:, h * KT + kt, :],
                                    start=(kt == 0), stop=(kt == E // 128 - 1))
                            nc.vector.tensor_scalar(
                                zT[:, h * 4 + m4, jh * 512:(jh + 1) * 512],
                                pz[:], b2[:, m4:m4 + 1], None, ALU.add)
                    # row norms for this half
                    pn = ps1.tile([1, 512], F32, tag="pn", bufs=2,
                                  name=f"pn{jh}")
                    for kt in range(KT):
                        sq = sqp.tile([128, 512], F32R, tag="sq")
                        zsl = zT[:, kt, jh * 512:(jh + 1) * 512]
                        nc.vector.tensor_tensor(sq[:], zsl, zsl, ALU.mult)
                        nc.tensor.matmul(pn[:], ones_r[:], sq[:],
                                         start=(kt == 0), stop=(kt == KT - 1))
                    nrm = small.tile([1, 512], F32, tag="nrm", name=f"nrm{jh}")
                    nc.scalar.activation(nrm[:], pn[:], AF.Sqrt)
                    nc.vector.tensor_scalar_max(nrm[:], nrm[:], EPS)
                    rn_strip = small.tile([1, 512], F32, tag="rns",
                                          name=f"rns{jh}")
                    nc.vector.reciprocal(rn_strip[:], nrm[:])
                    nc.sync.dma_start(rn_dram[None, jh * 512:(jh + 1) * 512],
                                      rn_strip[:])
                    nc.sync.dma_start(
                        rn_bc[:, jh * 512:(jh + 1) * 512],
                        rn_dram[None, jh * 512:(jh + 1) * 512]
                        .to_broadcast([128, 512]))
                    # normalize and ship this half
                    for kt in range(KT):
                        nc.vector.tensor_tensor(
                            znT[:, kt, jh * 512:(jh + 1) * 512],
                            zT[:, kt, jh * 512:(jh + 1) * 512],
                            rn_bc[:, jh * 512:(jh + 1) * 512], ALU.mult)
                    nc.sync.dma_start(
                        ag_in[jh].rearrange("(kt p) j -> p kt j", p=128),
                        znT[:, :, jh * 512:(jh + 1) * 512])
                    if no_collective:
                        for c in range(N_CORES):
                            nc.sync.dma_start(
                                ag_out[jh][c * D:(c + 1) * D, :], ag_in[jh][:])
                    else:
                        nc.gpsimd.collective_compute(
                            "AllGather", ALU.bypass,
                            replica_groups=[list(range(N_CORES))],
                            ins=[ag_in[jh][:]], outs=[ag_out[jh][:]])

            # ---- phase 2: blockwise cos-sim + fused exp/rowsum ----
            # slots 0..15: gathered col-tiles; 16..17: local block (dup of
            # own two slots, subtracted via osel at the end)
            rowsum = const.tile([128, MT, CT + 2], F32)
            pos_acc = const.tile([128, MT], F32)
            corr_acc = const.tile([128, MT], F32)
            nc.vector.memset(pos_acc[:], 0.0)

            def gemm_tile(pg, rhs_ap, m):
                for kt in range(KT):
                    nc.tensor.matmul(pg[:],
                                     znT[:, kt, m * 128:(m + 1) * 128],
                                     rhs_ap[:, kt, :],
                                     start=(kt == 0), stop=(kt == KT - 1))

            with tc.tile_pool(name="rhs", bufs=2) as rhsp, \
                 tc.tile_pool(name="junk", bufs=2) as junkp, \
                 tc.tile_pool(name="dtmp", bufs=4) as dtmpp, \
                 tc.tile_pool(name="ps2", bufs=1, space="PSUM") as ps2:

                # local block first — overlaps the collectives
                for nb in range(2):
                    for m in range(MT):
                        pg = ps2.tile([128, 512], F32, tag="pg", bufs=7,
                                      name=f"pgl{nb}_{m}")
                        gemm_tile(pg, znT[:, :, nb * 512:(nb + 1) * 512], m)
                        junk = junkp.tile([128, 512], F32, tag="junk")
                        nc.scalar.activation(
                            junk[:], pg[:], AF.Exp, scale=INV_T,
                            accum_out=rowsum[:, m, CT + nb:CT + nb + 1])
                        if nb == m // 4:
                            # self-similarity at compile-time position
                            off = (m % 4) * 128
                            jd = junkp.tile([128, 128], F32, tag="jd")
                            nc.vector.tensor_tensor(
                                jd[:], pg[:, off:off + 128], ident[:],
                                ALU.mult)
                            d = dtmpp.tile([128, 1], F32, tag="d")
                            nc.vector.reduce_sum(d[:], jd[:],
                                                 axis=mybir.AxisListType.X)
                            nc.scalar.activation(
                                corr_acc[:, m:m + 1], d[:], AF.Exp,
                                scale=INV_T)

                # gathered blocks, chunk by chunk
                for k in range(2):
                    for c2 in range(N_CORES):
                        ci = 2 * c2 + k
                        rhs = rhsp.tile([128, KT, 512], F32R, tag="rhs")
                        src = ag_out[k][c2 * D:(c2 + 1) * D, :].rearrange(
                            "(kt p) j -> p kt j", p=128)
                        nc.sync.dma_start(rhs[:], src)
                        for m in range(MT):
                            pg = ps2.tile([128, 512], F32, tag="pg", bufs=7,
                                          name=f"pg{ci}_{m}")
                            gemm_tile(pg, rhs, m)
                            junk = junkp.tile([128, 512], F32, tag="junk")
                            nc.scalar.activation(
                                junk[:], pg[:], AF.Exp, scale=INV_T,
                                accum_out=rowsum[:, m, ci:ci + 1])
                            # positive-pair logit (remote block diagonal)
                            if ci >= m // 4 and (ci - m // 4) % 2 == 0:
                                off = (m % 4) * 128
                                jd = junkp.tile([128, 128], F32, tag="jd")
                                nc.vector.tensor_tensor(
                                    jd[:], pg[:, off:off + 128], ident[:],
                                    ALU.mult)
                                d = dtmpp.tile([128, 1], F32, tag="d")
                                nc.vector.reduce_sum(d[:], jd[:],
                                                     axis=mybir.AxisListType.X)
                                tp = dtmpp.tile([128, 1], F32, tag="tp2")
                                nc.vector.tensor_scalar(
                                    tp[:], d[:], psel[:, m, ci:ci + 1], INV_T,
                                    ALU.mult, ALU.mult)
                                nc.vector.tensor_tensor(
                                    pos_acc[:, m:m + 1], pos_acc[:, m:m + 1],
                                    tp[:], ALU.add)

                # ---- finale: lse, nll, partial sum (batched over m) ----
                rs = small.tile([128, MT], F32)
                nc.vector.reduce_sum(rs[:], rowsum[:], axis=mybir.AxisListType.X)
                own = small.tile([128, MT, CT], F32)
                nc.vector.tensor_tensor(
                    own[:], rowsum[:, :, :CT],
                    osel[:, None, :].to_broadcast([128, MT, CT]), ALU.mult)
                ro = small.tile([128, MT], F32)
                nc.vector.reduce_sum(ro[:], own[:], axis=mybir.AxisListType.X)
                nc.vector.tensor_tensor(rs[:], rs[:], ro[:], ALU.subtract)
                nc.vector.tensor_tensor(rs[:], rs[:], corr_acc[:], ALU.subtract)
                lse = small.tile([128, MT], F32)
                nc.scalar.activation(lse[:], rs[:], AF.Ln)
                nll = small.tile([128, MT], F32)
                nc.vector.tensor_tensor(nll[:], lse[:], pos_acc[:], ALU.subtract)
                pf = ps2.tile([1, MT], F32, tag="pf", bufs=1)
                nc.tensor.matmul(pf[:], ones_f[:], nll[:], start=True, stop=True)
                fs = small.tile([1, MT], F32)
                nc.vector.tensor_copy(fs[:], pf[:])
                nc.sync.dma_start(out[:], fs[:])

    nc.compile()
    return nc


def _sel_masks(c):
    osel = np.zeros((128, CT), dtype=np.float32)
    osel[:, 2 * c] = 1.0
    osel[:, 2 * c + 1] = 1.0
    psel = np.zeros((128, MT, CT), dtype=np.float32)
    for m in range(MT):
        psel[:, m, 2 * ((c + 4) % N_CORES) + m // 4] = 1.0
    return osel, psel


def kernel(x, w, b):
    if "nc" not in _cached:
        _cached["nc"] = _build()
    nc = _cached["nc"]
    x = np.ascontiguousarray(np.asarray(x, dtype=np.float32))
    w = np.ascontiguousarray(np.asarray(w, dtype=np.float32))
    b = np.ascontiguousarray(np.asarray(b, dtype=np.float32))
    in_maps = []
    for c in range(N_CORES):
        osel, psel = _sel_masks(c)
        in_maps.append({
            "xs": np.ascontiguousarray(x[c * R:(c + 1) * R]),
            "w": w, "b": b, "osel": osel, "psel": psel,
        })
    res = run_bass_kernel_spmd(nc, in_maps, list(range(N_CORES)))
    total = 0.0
    for c in range(N_CORES):
        total += float(res.results[c]["out"].astype(np.float64).sum())
    return np.float32(total / B)


# revision 9
# speedup vs baseline: 1.2615x; 1.1362x over previous
"""Contrastive projection head loss on 8 Trainium2 NeuronCores.

Reference computation (B=8192, E=1024, P=512):
    z_codon = relu(x[:, :E]) @ w + b          # [B, P]
    z_amino = relu(x[:, E:]) @ w + b          # [B, P]
    z  = concat([z_codon, z_amino], axis=1)   # [B, 2P]
    zn = z / max(||z||, 1e-8)
    s  = (zn @ zn.T);  s[i,i] = -9e15;  s /= 0.1
    nll_i = -s[i, (i - B/2) % B] + logsumexp(s[i, :])
    out = mean(nll)

Distribution: data-parallel over B (1024 rows/core). Each core projects and
normalizes its rows (kept feature-major as zn^T — directly the K-major
operand of the similarity GEMM), all-gathers zn^T, then computes its
[1024 x 8192] block of the cosine-similarity matrix blockwise with a fused
exp+row-sum epilogue (ACT accum_out).

Latency hiding: phase 1 is pipelined by row-halves so the first of two
chunked AllGathers launches as soon as half the rows are projected and
normalized; the local diagonal block is computed straight from SBUF while
the collectives fly, and its duplicate contribution from the gathered pass
is subtracted via a per-core 0/1 mask ("osel") so the SPMD program stays
identical on all cores. The self-similarity term is removed by subtracting
its exp (extracted from the local block at compile-time-known positions);
the positive-pair logit is pulled from the block diagonal of the
(c+4) mod 8 column block, selected by the per-core "psel" mask.
Matmuls run in float32r (full-rate fp32 on the PE array).

Returns per-core partial sums [1, 8]; host sums and divides by B.
"""
import numpy as np

from concourse import mybir, tile, bacc
from concourse.bass_utils import run_bass_kernel_spmd
from concourse.masks import make_identity

N_CORES = 8
B = 8192
E = 1024          # embedding size (per half)
P = 512           # projection size
D = 2 * P         # z feature dim = 1024
R = B // N_CORES  # rows per core = 1024
KT = D // 128     # feature sub-tiles = 8
MT = R // 128     # row sub-tiles per core = 8
CT = B // 512     # global column tiles of 512 = 16
INV_T = 10.0      # 1 / temperature
EPS = 1e-8

F32 = mybir.dt.float32
F32R = mybir.dt.float32r
BF16 = mybir.dt.bfloat16
GDT = BF16        # dtype of gathered zn^T / similarity-GEMM operands
AF = mybir.ActivationFunctionType
ALU = mybir.AluOpType

_cached = {}


def _build(no_collective=False):
    nc = bacc.Bacc("TRN2", target_bir_lowering=False, debug=False,
                   enable_asserts=False, num_devices=N_CORES)
    x_in = nc.dram_tensor("xs", [R, 2 * E], F32, kind="ExternalInput").ap()
    w_in = nc.dram_tensor("w", [E, P], F32, kind="ExternalInput").ap()
    b_in = nc.dram_tensor("b", [P], F32, kind="ExternalInput").ap()
    osel_in = nc.dram_tensor("osel", [128, CT], F32, kind="ExternalInput").ap()
    psel_in = nc.dram_tensor("psel", [128, MT, CT], F32, kind="ExternalInput").ap()
    out = nc.dram_tensor("out", [1, MT], F32, kind="ExternalOutput").ap()

    with tile.TileContext(nc) as tc:
        with tc.tile_pool(name="const", bufs=1) as const, \
             tc.tile_pool(name="big", bufs=2) as big, \
             tc.tile_pool(name="small", bufs=1) as small, \
             tc.tile_pool(name="dram", bufs=1, space="DRAM") as dram:

            ident = const.tile([128, 128], F32)
            make_identity(nc, ident[:])
            ones_f = const.tile([128, 1], F32)
            nc.vector.memset(ones_f[:], 1.0)
            ones_r = const.tile([128, 1], F32R)
            nc.vector.tensor_copy(ones_r[:], ones_f[:])
            b2 = const.tile([128, P // 128], F32)
            nc.sync.dma_start(b2[:], b_in.rearrange("(mt p) -> p mt", p=128))
            osel = const.tile([128, CT], F32)
            nc.sync.dma_start(osel[:], osel_in[:])
            psel = const.tile([128, MT, CT], F32)
            nc.sync.dma_start(psel[:], psel_in[:])
            rn_bc = const.tile([128, R], F32)

            # w as [128, KT(=E/128), P] float32r — staged in a scoped pool
            w_r = const.tile([128, E // 128, P], F32R)
            with tc.tile_pool(name="wst", bufs=1) as wst:
                wstage = wst.tile([128, E // 128, P], F32, tag="wstage")
                nc.sync.dma_start(wstage[:],
                                  w_in.rearrange("(kt p) q -> p kt q", p=128))
                nc.vector.tensor_copy(w_r[:], wstage[:])

            # z^T feature-major, f32r; znT is the normalized copy
            zT = big.tile([128, KT, R], F32R, tag="z")
            znT = big.tile([128, KT, R], GDT, tag="z")
            ag_in = [dram.tile([D, 512], GDT, name=f"ag_in{k}")
                     for k in range(2)]
            ag_out = [dram.tile([N_CORES * D, 512], GDT, name=f"ag_out{k}",
                                addr_space="Local" if no_collective else "Shared")
                      for k in range(2)]
            rn_dram = dram.tile([R], F32)

            # ---- phase 1, pipelined over row-halves jh ----
            with tc.tile_pool(name="xrow", bufs=2) as xrowp, \
                 tc.tile_pool(name="xTp", bufs=2) as xTp, \
                 tc.tile_pool(name="sqp", bufs=2) as sqp, \
                 tc.tile_pool(name="ps1", bufs=2, space="PSUM") as ps1:
                for jh in range(2):
                    # transpose rows of this half (both x halves), with relu
                    xT = xTp.tile([128, 2 * KT, 512], F32R, tag="xT",
                                  name=f"xT{jh}")
                    for r in range(4):
                        rg = jh * 4 + r
                        xrow = xrowp.tile([128, 2 * E], F32, tag="xrow")
                        nc.sync.dma_start(xrow[:],
                                          x_in[rg * 128:(rg + 1) * 128, :])
                        for cg in range(2 * E // 512):
                            pt = ps1.tile([128, 4, 128], F32, tag="tp", bufs=3)
                            for q in range(4):
                                ct = cg * 4 + q
                                nc.tensor.transpose(
                                    pt[:, q, :],
                                    xrow[:, ct * 128:(ct + 1) * 128],
                                    ident[:])
                            nc.vector.tensor_scalar_max(
                                xT[:, cg * 4:(cg + 1) * 4,
                                   r * 128:(r + 1) * 128].rearrange(
                                       "p c j -> p c j"),
                                pt[:], 0.0)
                    # project this half: zT[:, h*4+m4, jh*512:...]
                    for h in range(2):
                        for m4 in range(P // 128):
                            pz = ps1.tile([128, 512], F32, tag="pz", bufs=2)
                            for kt in range(E // 128):
                                nc.tensor.matmul(
                                    pz[:],
                                    w_r[:, kt, m4 * 128:(m4 + 1) * 128],
                                    xT[:, h * KT + kt, :],
                                    start=(kt == 0), stop=(kt == E // 128 - 1))
                            nc.vector.tensor_scalar(
                                zT[:, h * 4 + m4, jh * 512:(jh + 1) * 512],
                                pz[:], b2[:, m4:m4 + 1], None, ALU.add)
                    # row norms for this half
                    pn = ps1.tile([1, 512], F32, tag="pn", bufs=2,
                                  name=f"pn{jh}")
                    for kt in range(KT):
                        sq = sqp.tile([128, 512], F32R, tag="sq")
                        zsl = zT[:, kt, jh * 512:(jh + 1) * 512]
                        nc.vector.tensor_tensor(sq[:], zsl, zsl, ALU.mult)
                        nc.tensor.matmul(pn[:], ones_r[:], sq[:],
                                         start=(kt == 0), stop=(kt == KT - 1))
                    nrm = small.tile([1, 512], F32, tag="nrm", name=f"nrm{jh}")
                    nc.scalar.activation(nrm[:], pn[:], AF.Sqrt)
                    nc.vector.tensor_scalar_max(nrm[:], nrm[:], EPS)
                    rn_strip = small.tile([1, 512], F32, tag="rns",
                                          name=f"rns{jh}")
                    nc.vector.reciprocal(rn_strip[:], nrm[:])
                    nc.sync.dma_start(rn_dram[None, jh * 512:(jh + 1) * 512],
                                      rn_strip[:])
                    nc.sync.dma_start(
                        rn_bc[:, jh * 512:(jh + 1) * 512],
                        rn_dram[None, jh * 512:(jh + 1) * 512]
                        .to_broadcast([128, 512]))
                    # normalize and ship this half
                    for kt in range(KT):
                        nc.vector.tensor_tensor(
                            znT[:, kt, jh * 512:(jh + 1) * 512],
                            zT[:, kt, jh * 512:(jh + 1) * 512],
                            rn_bc[:, jh * 512:(jh + 1) * 512], ALU.mult)
                    nc.sync.dma_start(
                        ag_in[jh].rearrange("(kt p) j -> p kt j", p=128),
                        znT[:, :, jh * 512:(jh + 1) * 512])
                    if no_collective:
                        for c in range(N_CORES):
                            nc.sync.dma_start(
                                ag_out[jh][c * D:(c + 1) * D, :], ag_in[jh][:])
                    else:
                        nc.gpsimd.collective_compute(
                            "AllGather", ALU.bypass,
                            replica_groups=[list(range(N_CORES))],
                            ins=[ag_in[jh][:]], outs=[ag_out[jh][:]])

            # ---- phase 2: blockwise cos-sim + fused exp/rowsum ----
            # slots 0..15: gathered col-tiles; 16..17: local block (dup of
            # own two slots, subtracted via osel at the end)
            rowsum = const.tile([128, MT, CT + 2], F32)
            pos_acc = const.tile([128, MT], F32)
            corr_acc = const.tile([128, MT], F32)
            nc.vector.memset(pos_acc[:], 0.0)

            def gemm_tile(pg, rhs_ap, m):
                for kt in range(KT):
                    nc.tensor.matmul(pg[:],
                                     znT[:, kt, m * 128:(m + 1) * 128],
                                     rhs_ap[:, kt, :],
                                     start=(kt == 0), stop=(kt == KT - 1))

            with tc.tile_pool(name="rhs", bufs=2) as rhsp, \
                 tc.tile_pool(name="junk", bufs=2) as junkp, \
                 tc.tile_pool(name="dtmp", bufs=4) as dtmpp, \
                 tc.tile_pool(name="ps2", bufs=1, space="PSUM") as ps2:

                # local block first — overlaps the collectives
                for nb in range(2):
                    for m in range(MT):
                        pg = ps2.tile([128, 512], F32, tag="pg", bufs=7,
                                      name=f"pgl{nb}_{m}")
                        gemm_tile(pg, znT[:, :, nb * 512:(nb + 1) * 512], m)
                        junk = junkp.tile([128, 512], F32, tag="junk")
                        nc.scalar.activation(
                            junk[:], pg[:], AF.Exp, scale=INV_T,
                            accum_out=rowsum[:, m, CT + nb:CT + nb + 1])
                        if nb == m // 4:
                            # self-similarity at compile-time position
                            off = (m % 4) * 128
                            jd = junkp.tile([128, 128], F32, tag="jd")
                            nc.vector.tensor_tensor(
                                jd[:], pg[:, off:off + 128], ident[:],
                                ALU.mult)
                            d = dtmpp.tile([128, 1], F32, tag="d")
                            nc.vector.reduce_sum(d[:], jd[:],
                                                 axis=mybir.AxisListType.X)
                            nc.scalar.activation(
                                corr_acc[:, m:m + 1], d[:], AF.Exp,
                                scale=INV_T)

                # gathered blocks, chunk by chunk
                for k in range(2):
                    for c2 in range(N_CORES):
                        ci = 2 * c2 + k
                        rhs = rhsp.tile([128, KT, 512], GDT, tag="rhs")
                        src = ag_out[k][c2 * D:(c2 + 1) * D, :].rearrange(
                            "(kt p) j -> p kt j", p=128)
                        nc.sync.dma_start(rhs[:], src)
                        for m in range(MT):
                            pg = ps2.tile([128, 512], F32, tag="pg", bufs=7,
                                          name=f"pg{ci}_{m}")
                            gemm_tile(pg, rhs, m)
                            junk = junkp.tile([128, 512], F32, tag="junk")
                            nc.scalar.activation(
                                junk[:], pg[:], AF.Exp, scale=INV_T,
                                accum_out=rowsum[:, m, ci:ci + 1])
                            # positive-pair logit (remote block diagonal)
                            if ci >= m // 4 and (ci - m // 4) % 2 == 0:
                                off = (m % 4) * 128
                                jd = junkp.tile([128, 128], F32, tag="jd")
                                nc.vector.tensor_tensor(
                                    jd[:], pg[:, off:off + 128], ident[:],
                                    ALU.mult)
                                d = dtmpp.tile([128, 1], F32, tag="d")
                                nc.vector.reduce_sum(d[:], jd[:],
                                                     axis=mybir.AxisListType.X)
                                tp = dtmpp.tile([128, 1], F32, tag="tp2")
                                nc.vector.tensor_scalar(
                                    tp[:], d[:], psel[:, m, ci:ci + 1], INV_T,
                                    ALU.mult, ALU.mult)
                                nc.vector.tensor_tensor(
                                    pos_acc[:, m:m + 1], pos_acc[:, m:m + 1],
                                    tp[:], ALU.add)

                # ---- finale: lse, nll, partial sum (batched over m) ----
                rs = small.tile([128, MT], F32)
                nc.vector.reduce_sum(rs[:], rowsum[:], axis=mybir.AxisListType.X)
                own = small.tile([128, MT, CT], F32)
                nc.vector.tensor_tensor(
                    own[:], rowsum[:, :, :CT],
                    osel[:, None, :].to_broadcast([128, MT, CT]), ALU.mult)
                ro = small.tile([128, MT], F32)
                nc.vector.reduce_sum(ro[:], own[:], axis=mybir.AxisListType.X)
                nc.vector.tensor_tensor(rs[:], rs[:], ro[:], ALU.subtract)
                nc.vector.tensor_tensor(rs[:], rs[:], corr_acc[:], ALU.subtract)
                lse = small.tile([128, MT], F32)
                nc.scalar.activation(lse[:], rs[:], AF.Ln)
                nll = small.tile([128, MT], F32)
                nc.vector.tensor_tensor(nll[:], lse[:], pos_acc[:], ALU.subtract)
                pf = ps2.tile([1, MT], F32, tag="pf", bufs=1)
                nc.tensor.matmul(pf[:], ones_f[:], nll[:], start=True, stop=True)
                fs = small.tile([1, MT], F32)
                nc.vector.tensor_copy(fs[:], pf[:])
                nc.sync.dma_start(out[:], fs[:])

    nc.compile()
    return nc


def _sel_masks(c):
    osel = np.zeros((128, CT), dtype=np.float32)
    osel[:, 2 * c] = 1.0
    osel[:, 2 * c + 1] = 1.0
    psel = np.zeros((128, MT, CT), dtype=np.float32)
    for m in range(MT):
        psel[:, m, 2 * ((c + 4) % N_CORES) + m // 4] = 1.0
    return osel, psel


def kernel(x, w, b):
    if "nc" not in _cached:
        _cached["nc"] = _build()
    nc = _cached["nc"]
    x = np.ascontiguousarray(np.asarray(x, dtype=np.float32))
    w = np.ascontiguousarray(np.asarray(w, dtype=np.float32))
    b = np.ascontiguousarray(np.asarray(b, dtype=np.float32))
    in_maps = []
    for c in range(N_CORES):
        osel, psel = _sel_masks(c)
        in_maps.append({
            "xs": np.ascontiguousarray(x[c * R:(c + 1) * R]),
            "w": w, "b": b, "osel": osel, "psel": psel,
        })
    res = run_bass_kernel_spmd(nc, in_maps, list(range(N_CORES)))
    total = 0.0
    for c in range(N_CORES):
        total += float(res.results[c]["out"].astype(np.float64).sum())
    return np.float32(total / B)


# revision 10
# speedup vs baseline: 1.7423x; 1.3812x over previous
"""Contrastive projection head loss on 8 Trainium2 NeuronCores.

Reference computation (B=8192, E=1024, P=512):
    z_codon = relu(x[:, :E]) @ w + b          # [B, P]
    z_amino = relu(x[:, E:]) @ w + b          # [B, P]
    z  = concat([z_codon, z_amino], axis=1)   # [B, 2P]
    zn = z / max(||z||, 1e-8)
    s  = (zn @ zn.T);  s[i,i] = -9e15;  s /= 0.1
    nll_i = -s[i, (i - B/2) % B] + logsumexp(s[i, :])
    out = mean(nll)

Distribution: data-parallel over B (1024 rows/core). Each core projects and
normalizes its rows (kept feature-major as zn^T — directly the K-major
operand of the similarity GEMM) and all-gathers zn^T (bf16) in two column
chunks, pipelined with phase 1 by row-halves.

The similarity matrix is symmetric, so each core computes only the block
column range d = 0..4 (its own rows against cores c..c+4 mod 8), halving
the GEMM. Blocks d=1..3 additionally produce column sums of exp(s/T) (one
PE ones-matmul per tile) which are routed to the owning cores with a
ReduceScatter; block d=4 is computed by both endpoints (row sums only), so
every row's logsumexp denominator is covered exactly once. Remote operands
are addressed with partition-id-derived dynamic DMA offsets, keeping the
SPMD program identical on all cores. The self-similarity term is removed
by subtracting its exp (block d=0, compile-time positions); the
positive-pair logit is the block diagonal of the d=4 block.

Returns per-core partial sums [1, 8]; host sums and divides by B.
"""
import numpy as np

from concourse import bass, mybir, tile, bacc
from concourse.bass_utils import run_bass_kernel_spmd
from concourse.masks import make_identity

N_CORES = 8
B = 8192
E = 1024          # embedding size (per half)
P = 512           # projection size
D = 2 * P         # z feature dim = 1024
R = B // N_CORES  # rows per core = 1024
KT = D // 128     # feature sub-tiles = 8
MT = R // 128     # row sub-tiles per core = 8
INV_T = 10.0      # 1 / temperature
EPS = 1e-8

F32 = mybir.dt.float32
F32R = mybir.dt.float32r
BF16 = mybir.dt.bfloat16
GDT = BF16        # dtype of gathered zn^T / similarity-GEMM operands
AF = mybir.ActivationFunctionType
ALU = mybir.AluOpType

NSLOT = 10        # rowsum slots: 2 local (d=0) + 8 remote (d=1..4, k=0..1)

_cached = {}


def _build(no_collective=False):
    nc = bacc.Bacc("TRN2", target_bir_lowering=False, debug=False,
                   enable_asserts=False, num_devices=N_CORES)
    x_in = nc.dram_tensor("xs", [R, 2 * E], F32, kind="ExternalInput").ap()
    w_in = nc.dram_tensor("w", [E, P], F32, kind="ExternalInput").ap()
    b_in = nc.dram_tensor("b", [P], F32, kind="ExternalInput").ap()
    out = nc.dram_tensor("out", [1, MT], F32, kind="ExternalOutput").ap()

    with tile.TileContext(nc) as tc:
        with tc.tile_pool(name="const", bufs=1) as const, \
             tc.tile_pool(name="big", bufs=2) as big, \
             tc.tile_pool(name="small", bufs=1) as small, \
             tc.tile_pool(name="dram", bufs=1, space="DRAM") as dram:

            ident = const.tile([128, 128], F32)
            make_identity(nc, ident[:])
            ones_f = const.tile([128, 1], F32)
            nc.vector.memset(ones_f[:], 1.0)
            ones_r = const.tile([128, 1], F32R)
            nc.vector.tensor_copy(ones_r[:], ones_f[:])
            ones_b = const.tile([128, 1], BF16)
            nc.vector.tensor_copy(ones_b[:], ones_f[:])
            b2 = const.tile([128, P // 128], F32)
            nc.sync.dma_start(b2[:], b_in.rearrange("(mt p) -> p mt", p=128))
            rn_bc = const.tile([128, R], F32)

            # w as [128, KT(=E/128), P] float32r — staged in a scoped pool
            w_r = const.tile([128, E // 128, P], F32R)
            with tc.tile_pool(name="wst", bufs=1) as wst:
                wstage = wst.tile([128, E // 128, P], F32, tag="wstage")
                nc.sync.dma_start(wstage[:],
                                  w_in.rearrange("(kt p) q -> p kt q", p=128))
                nc.vector.tensor_copy(w_r[:], wstage[:])

            # z^T feature-major, f32r; znT is the normalized bf16 copy
            zT = big.tile([128, KT, R], F32R, tag="z")
            znT = big.tile([128, KT, R], GDT, tag="z")
            ag_in = [dram.tile([D, 512], GDT, name=f"ag_in{k}")
                     for k in range(2)]
            ag_out = [dram.tile([N_CORES * D, 512], GDT, name=f"ag_out{k}",
                                addr_space="Local" if no_collective else "Shared")
                      for k in range(2)]
            rn_dram = dram.tile([R], F32)
            rs_in = dram.tile([N_CORES, R], F32)
            rs_out = dram.tile([R], F32)

            # ---- phase 1, pipelined over row-halves jh ----
            with tc.tile_pool(name="xrow", bufs=2) as xrowp, \
                 tc.tile_pool(name="xTp", bufs=2) as xTp, \
                 tc.tile_pool(name="sqp", bufs=2) as sqp, \
                 tc.tile_pool(name="ps1", bufs=2, space="PSUM") as ps1:
                for jh in range(2):
                    # transpose rows of this half (both x halves), with relu
                    xT = xTp.tile([128, 2 * KT, 512], F32R, tag="xT",
                                  name=f"xT{jh}")
                    for r in range(4):
                        rg = jh * 4 + r
                        xrow = xrowp.tile([128, 2 * E], F32, tag="xrow")
                        nc.sync.dma_start(xrow[:],
                                          x_in[rg * 128:(rg + 1) * 128, :])
                        for cg in range(2 * E // 512):
                            pt = ps1.tile([128, 4, 128], F32, tag="tp", bufs=3)
                            for q in range(4):
                                ct = cg * 4 + q
                                nc.tensor.transpose(
                                    pt[:, q, :],
                                    xrow[:, ct * 128:(ct + 1) * 128],
                                    ident[:])
                            nc.vector.tensor_scalar_max(
                                xT[:, cg * 4:(cg + 1) * 4,
                                   r * 128:(r + 1) * 128],
                                pt[:], 0.0)
                    # project this half: zT[:, h*4+m4, jh*512:...]
                    for h in range(2):
                        for m4 in range(P // 128):
                            pz = ps1.tile([128, 512], F32, tag="pz", bufs=2)
                            for kt in range(E // 128):
                                nc.tensor.matmul(
                                    pz[:],
                                    w_r[:, kt, m4 * 128:(m4 + 1) * 128],
                                    xT[:, h * KT + kt, :],
                                    start=(kt == 0), stop=(kt == E // 128 - 1))
                            nc.vector.tensor_scalar(
                                zT[:, h * 4 + m4, jh * 512:(jh + 1) * 512],
                                pz[:], b2[:, m4:m4 + 1], None, ALU.add)
                    # row norms for this half
                    pn = ps1.tile([1, 512], F32, tag="pn", bufs=2,
                                  name=f"pn{jh}")
                    for kt in range(KT):
                        sq = sqp.tile([128, 512], F32R, tag="sq")
                        zsl = zT[:, kt, jh * 512:(jh + 1) * 512]
                        nc.vector.tensor_tensor(sq[:], zsl, zsl, ALU.mult)
                        nc.tensor.matmul(pn[:], ones_r[:], sq[:],
                                         start=(kt == 0), stop=(kt == KT - 1))
                    nrm = small.tile([1, 512], F32, tag="nrm", name=f"nrm{jh}")
                    nc.scalar.activation(nrm[:], pn[:], AF.Sqrt)
                    nc.vector.tensor_scalar_max(nrm[:], nrm[:], EPS)
                    rn_strip = small.tile([1, 512], F32, tag="rns",
                                          name=f"rns{jh}")
                    nc.vector.reciprocal(rn_strip[:], nrm[:])
                    nc.sync.dma_start(rn_dram[None, jh * 512:(jh + 1) * 512],
                                      rn_strip[:])
                    nc.sync.dma_start(
                        rn_bc[:, jh * 512:(jh + 1) * 512],
                        rn_dram[None, jh * 512:(jh + 1) * 512]
                        .to_broadcast([128, 512]))
                    # normalize and ship this half
                    for kt in range(KT):
                        nc.vector.tensor_tensor(
                            znT[:, kt, jh * 512:(jh + 1) * 512],
                            zT[:, kt, jh * 512:(jh + 1) * 512],
                            rn_bc[:, jh * 512:(jh + 1) * 512], ALU.mult)
                    nc.sync.dma_start(
                        ag_in[jh].rearrange("(kt p) j -> p kt j", p=128),
                        znT[:, :, jh * 512:(jh + 1) * 512])
                    if no_collective:
                        for c in range(N_CORES):
                            nc.sync.dma_start(
                                ag_out[jh][c * D:(c + 1) * D, :], ag_in[jh][:])
                    else:
                        nc.gpsimd.collective_compute(
                            "AllGather", ALU.bypass,
                            replica_groups=[list(range(N_CORES))],
                            ins=[ag_in[jh][:]], outs=[ag_out[jh][:]])

            # ---- phase 2: symmetric blockwise cos-sim ----
            rowsum = const.tile([128, MT, NSLOT], F32)
            pos_acc = const.tile([128, MT], F32)
            corr_acc = const.tile([128, MT], F32)

            pid = nc.sync.partition_id()

            def gemm_tile(pg, rhs_ap, m):
                for kt in range(KT):
                    nc.tensor.matmul(pg[:],
                                     znT[:, kt, m * 128:(m + 1) * 128],
                                     rhs_ap[:, kt, :],
                                     start=(kt == 0), stop=(kt == KT - 1))

            with tc.tile_pool(name="rhs", bufs=2) as rhsp, \
                 tc.tile_pool(name="junk", bufs=3) as junkp, \
                 tc.tile_pool(name="dtmp", bufs=4) as dtmpp, \
                 tc.tile_pool(name="ps2", bufs=1, space="PSUM") as ps2:

                # zero the ReduceScatter input (slots we don't write must be 0)
                zb = small.tile([N_CORES, R], F32)
                nc.vector.memset(zb[:], 0.0)
                nc.sync.dma_start(rs_in[:], zb[:])

                # local block d=0 first — overlaps the collectives
                for nb in range(2):
                    for m in range(MT):
                        pg = ps2.tile([128, 512], F32, tag="pg", bufs=5,
                                      name=f"pgl{nb}_{m}")
                        gemm_tile(pg, znT[:, :, nb * 512:(nb + 1) * 512], m)
                        junk = junkp.tile([128, 512], BF16, tag="junk")
                        nc.scalar.activation(
                            junk[:], pg[:], AF.Exp, scale=INV_T,
                            accum_out=rowsum[:, m, nb:nb + 1])
                        if nb == m // 4:
                            # self-similarity at compile-time position
                            off = (m % 4) * 128
                            jd = junkp.tile([128, 128], F32, tag="jd")
                            nc.vector.tensor_tensor(
                                jd[:], pg[:, off:off + 128], ident[:],
                                ALU.mult)
                            d = dtmpp.tile([128, 1], F32, tag="d")
                            nc.vector.reduce_sum(d[:], jd[:],
                                                 axis=mybir.AxisListType.X)
                            nc.scalar.activation(
                                corr_acc[:, m:m + 1], d[:], AF.Exp,
                                scale=INV_T)

                # remote blocks d = 1..4, per gathered chunk k
                for k in range(2):
                    for dd in range(1, 5):
                        slot = 2 + (dd - 1) * 2 + k
                        row0 = ((pid + dd) % N_CORES) * D
                        rhs = rhsp.tile([128, KT, 512], GDT, tag="rhs")
                        src = ag_out[k][bass.ds(row0, D), :].rearrange(
                            "(kt p) j -> p kt j", p=128)
                        nc.sync.dma_start(rhs[:], src)
                        cs = None
                        if dd < 4:
                            cs = ps2.tile([1, 512], F32, tag="cs", bufs=2,
                                          name=f"cs{k}_{dd}")
                        for m in range(MT):
                            pg = ps2.tile([128, 512], F32, tag="pg", bufs=5,
                                          name=f"pg{k}_{dd}_{m}")
                            gemm_tile(pg, rhs, m)
                            junk = junkp.tile([128, 512], BF16, tag="junk")
                            nc.scalar.activation(
                                junk[:], pg[:], AF.Exp, scale=INV_T,
                                accum_out=rowsum[:, m, slot:slot + 1])
                            if dd < 4:
                                nc.tensor.matmul(cs[:], ones_b[:], junk[:],
                                                 start=(m == 0),
                                                 stop=(m == MT - 1))
                            if dd == 4 and k == m // 4:
                                # positive-pair logit on the block diagonal
                                off = (m % 4) * 128
                                jd = junkp.tile([128, 128], F32, tag="jd")
                                nc.vector.tensor_tensor(
                                    jd[:], pg[:, off:off + 128], ident[:],
                                    ALU.mult)
                                dpos = dtmpp.tile([128, 1], F32, tag="dp")
                                nc.vector.reduce_sum(
                                    dpos[:], jd[:], axis=mybir.AxisListType.X)
                                nc.vector.tensor_scalar_mul(
                                    pos_acc[:, m:m + 1], dpos[:], INV_T)
                        if dd < 4:
                            # ship this block's column sums to core (c+dd)
                            css = dtmpp.tile([1, 512], F32, tag="css",
                                             name=f"css{k}_{dd}")
                            nc.vector.tensor_copy(css[:], cs[:])
                            nc.sync.dma_start(
                                rs_in[bass.ds((pid + dd) % N_CORES, 1),
                                      k * 512:(k + 1) * 512],
                                css[:])

                # sum exchanged column contributions
                if no_collective:
                    nc.sync.dma_start(rs_out[None, :], rs_in[0:1, :])
                else:
                    nc.gpsimd.collective_compute(
                        "ReduceScatter", ALU.add,
                        replica_groups=[list(range(N_CORES))],
                        ins=[rs_in[:]], outs=[rs_out[:]])

                # ---- finale: lse, nll, partial sum (batched over m) ----
                rs = small.tile([128, MT], F32)
                nc.vector.reduce_sum(rs[:], rowsum[:],
                                     axis=mybir.AxisListType.X)
                rcv = small.tile([128, MT], F32)
                nc.sync.dma_start(rcv[:], rs_out.rearrange("(m p) -> p m", p=128))
                nc.vector.tensor_tensor(rs[:], rs[:], rcv[:], ALU.add)
                nc.vector.tensor_tensor(rs[:], rs[:], corr_acc[:], ALU.subtract)
                lse = small.tile([128, MT], F32)
                nc.scalar.activation(lse[:], rs[:], AF.Ln)
                nll = small.tile([128, MT], F32)
                nc.vector.tensor_tensor(nll[:], lse[:], pos_acc[:], ALU.subtract)
                pf = ps2.tile([1, MT], F32, tag="pf", bufs=1)
                nc.tensor.matmul(pf[:], ones_f[:], nll[:], start=True, stop=True)
                fs = small.tile([1, MT], F32)
                nc.vector.tensor_copy(fs[:], pf[:])
                nc.sync.dma_start(out[:], fs[:])

    nc.compile()
    return nc


def kernel(x, w, b):
    if "nc" not in _cached:
        _cached["nc"] = _build()
    nc = _cached["nc"]
    x = np.ascontiguousarray(np.asarray(x, dtype=np.float32))
    w = np.ascontiguousarray(np.asarray(w, dtype=np.float32))
    b = np.ascontiguousarray(np.asarray(b, dtype=np.float32))
    in_maps = [{
        "xs": np.ascontiguousarray(x[c * R:(c + 1) * R]),
        "w": w, "b": b,
    } for c in range(N_CORES)]
    res = run_bass_kernel_spmd(nc, in_maps, list(range(N_CORES)))
    total = 0.0
    for c in range(N_CORES):
        total += float(res.results[c]["out"].astype(np.float64).sum())
    return np.float32(total / B)
